# revision 1
# baseline (speedup 1.0000x reference)
"""Trainium2 Bass kernel for nn_MultiHeadCausalTensionLayer.

Reference computation (B=1, T=2048, D=1024, H=16, HD=64, WN=64):
  q,k,v = x@wq, x@wk, x@wv  (per-head RoPE on q,k)
  scores[t,h,w] = q[t,h]·k[t-64+w,h] / 8          (w in [0,64), causal window)
  tau = sigmoid(scores) * causal_mask
  msg = (tau @ window_v) / clip(sum_w tau, 1e-6)
  out = rms_norm(x + msg.flat @ wo) * norm_scale

Sharding: sequence-parallel over T across 8 cores (256 rows each) with a
64-row halo; the halo is materialized host-side (zero-padded for core 0),
so each core's program is identical, fully local, and needs no collectives.

Matmuls run in fp32r (fp32 rounded to 11 mantissa bits; 4x the fp32 rate
at free-dim >= 256). Weights are pre-rounded on the host; on-chip matmul
operands are produced with float32r output dtype by ACT/DVE copies.
"""

import numpy as np

import concourse.bass as bass
import concourse.mybir as mybir
import concourse.tile as tile
from concourse import bacc, bass_utils

# Problem constants (hardcoded per harness contract).
B, T, D = 1, 2048, 1024
H, HD, WN = 16, 64, 64
ROPE_BASE = 10000.0
EPS = 1e-6
NCORES = 8
TLOC = T // NCORES          # 256 rows per core
TEXT = TLOC + WN            # 320 rows incl. halo
P = 128
KCH = D // P                # 8 contraction chunks
MCH = D // P                # 8 output chunks
NKB = TEXT // P + (1 if TEXT % P else 0)  # 3 key blocks (128,128,64)

f32 = mybir.dt.float32
f32r = mybir.dt.float32r


def _round_fp32r(x: np.ndarray) -> np.ndarray:
    """Round fp32 to the fp32r grid (11 mantissa bits, RNE)."""
    u = np.ascontiguousarray(x, dtype=np.float32).view(np.uint32)
    lsb = (u >> 12) & 1
    r = (u + 0x7FF + lsb) & np.uint32(0xFFFFF000)
    return r.view(np.float32)


def _build_program(loop_reps=None):
    nc = bacc.Bacc("TRN2", target_bir_lowering=False, debug=False)

    def din(name, shape, dt):
        return nc.dram_tensor(name, list(shape), dt, kind="ExternalInput").ap()

    x_halo = din("x_halo", (TEXT, D), f32)
    x_own = din("x_own", (TLOC, D), f32)
    wq_d = din("wq_r", (D, D), f32r)
    wk_d = din("wk_r", (D, D), f32r)
    wv_d = din("wv_r", (D, D), f32r)
    wo_d = din("wo_r", (D, D), f32r)
    ident_d = din("ident", (P, P), f32)
    rotT_d = din("rotT", (P, P), f32r)
    cosq_d = din("cosq", (P, TLOC), f32)
    sinq_d = din("sinq", (P, TLOC), f32)
    cosk_d = din("cosk", (P, TEXT), f32)
    sink_d = din("sink", (P, TEXT), f32)
    masks_d = din("masks", (NKB, P, TLOC), f32r)
    sel2_d = din("sel2", (1, 2, P), f32r)
    ones_d = din("ones_col", (P, 1), f32r)
    nsc_d = din("norm_scale", (P, D), f32)
    y_d = nc.dram_tensor("y", [TLOC, D], f32, kind="ExternalOutput").ap()

    with tile.TileContext(nc) as tc:
        from contextlib import ExitStack
        with ExitStack() as ctx:
            if loop_reps is not None:
                loop = ctx.enter_context(tc.For_i(0, loop_reps, 1))
            sb = ctx.enter_context(tc.tile_pool(name="sb", bufs=1))
            sbw = ctx.enter_context(tc.tile_pool(name="sbw", bufs=2))
            sbr = ctx.enter_context(tc.tile_pool(name="sbr", bufs=3))
            sbt = ctx.enter_context(tc.tile_pool(name="sbt", bufs=6))
            pp = ctx.enter_context(tc.tile_pool(name="pp", bufs=2, space="PSUM"))
            psc = ctx.enter_context(tc.tile_pool(name="psc", bufs=2, space="PSUM"))
            pmsg = ctx.enter_context(tc.tile_pool(name="pmsg", bufs=1, space="PSUM"))
            pms = ctx.enter_context(tc.tile_pool(name="pms", bufs=1, space="PSUM"))

            # ---- constant / input loads ----
            ident_t = sb.tile([P, P], f32, tag="ident")
            nc.sync.dma_start(ident_t[:], ident_d)
            rot_t = sb.tile([P, P], f32r, tag="rot")
            nc.sync.dma_start(rot_t[:], rotT_d)
            cq_t = sb.tile([P, TLOC], f32, tag="cq")
            sq_t = sb.tile([P, TLOC], f32, tag="sq")
            ck_t = sb.tile([P, TEXT], f32, tag="ck")
            sk_t = sb.tile([P, TEXT], f32, tag="sk")
            nc.sync.dma_start(cq_t[:], cosq_d)
            nc.sync.dma_start(sq_t[:], sinq_d)
            nc.sync.dma_start(ck_t[:], cosk_d)
            nc.sync.dma_start(sk_t[:], sink_d)
            mask_t = sb.tile([P, NKB, TLOC], f32r, tag="mask")
            for kb in range(NKB):
                nc.sync.dma_start(mask_t[:, kb, :], masks_d[kb])
            sel_t = sb.tile([1, 2, P], f32r, tag="sel")
            nc.sync.dma_start(sel_t[:], sel2_d)
            ones_t = sb.tile([P, 1], f32r, tag="ones")
            nc.sync.dma_start(ones_t[:], ones_d)
            nsc_t = sb.tile([P, D], f32, tag="nsc")
            nc.sync.dma_start(nsc_t[:], nsc_d)

            # x natural layout (for transposes + residual)
            xe_t = sbw.tile([P, NKB, D], f32, tag="w", name="xe_t")
            nc.sync.dma_start(xe_t[:, 0, :], x_halo[0:P])
            nc.sync.dma_start(xe_t[:, 1, :], x_halo[P:2 * P])
            nc.sync.dma_start(xe_t[0:TEXT - 2 * P, 2, :], x_halo[2 * P:TEXT])
            xo_t = sb.tile([P, 2, D], f32, tag="xo")
            nc.sync.dma_start(xo_t[:], x_own.rearrange("(c p) d -> p c d", p=P))

            # ---- transpose x -> xT [dout, text] (fp32 PE transpose, f32r out) ----
            xT_t = sb.tile([P, KCH, TEXT], f32r, tag="xT")
            for tc3 in range(NKB):
                rows = P if tc3 < 2 else TEXT - 2 * P
                for dc in range(KCH):
                    pt = pp.tile([P, 512], f32, tag="pp")
                    nc.tensor.transpose(
                        pt[:, 0:rows],
                        xe_t[0:rows, tc3, dc * P:(dc + 1) * P],
                        ident_t[0:rows, 0:rows],
                    )
                    nc.scalar.activation(
                        xT_t[:, dc, tc3 * P:tc3 * P + rows], pt[:, 0:rows],
                        mybir.ActivationFunctionType.Copy,
                    )

            # ---- weight tiles (streamed, one resident at a time) ----
            def load_w(wd):
                wt = sbw.tile([P, KCH, D], f32r, tag="w")
                for k in range(KCH):
                    nc.sync.dma_start(wt[:, k, :], wd[k * P:(k + 1) * P])
                return wt

            # ---- q/k projections (transposed orientation) + RoPE ----
            def proj_T(wt, ncols, col_off, cos_t, sin_t, out_tag):
                """out[dout, t] = w.T @ x.T, then RoPE; returns [P, MCH, ncols] f32r."""
                outT = sb.tile([P, MCH, ncols], f32r, tag=out_tag)
                for m in range(MCH):
                    pq = pp.tile([P, 512], f32, tag="pp")
                    for k in range(KCH):
                        nc.tensor.matmul(
                            pq[:, 0:ncols],
                            wt[:, k, m * P:(m + 1) * P],
                            xT_t[:, k, col_off:col_off + ncols],
                            start=(k == 0), stop=(k == KCH - 1),
                        )
                    a_t = sbr.tile([P, TEXT], f32r, tag="projchunk")
                    nc.scalar.activation(a_t[:, 0:ncols], pq[:, 0:ncols],
                                         mybir.ActivationFunctionType.Copy)
                    pr = pp.tile([P, 512], f32, tag="pp")
                    nc.tensor.matmul(pr[:, 0:ncols], rot_t[:], a_t[:, 0:ncols],
                                     start=True, stop=True)
                    t1 = sbr.tile([P, TEXT], f32, tag="ropescratch")
                    nc.gpsimd.tensor_tensor(t1[:, 0:ncols], a_t[:, 0:ncols],
                                            cos_t[:], op=mybir.AluOpType.mult)
                    t2 = sbr.tile([P, TEXT], f32, tag="ropescratch2")
                    nc.vector.tensor_tensor(t2[:, 0:ncols], pr[:, 0:ncols],
                                            sin_t[:], op=mybir.AluOpType.mult)
                    nc.vector.tensor_tensor(outT[:, m, :], t1[:, 0:ncols],
                                            t2[:, 0:ncols], op=mybir.AluOpType.add)
                return outT

            wq_t = load_w(wq_d)
            qT = proj_T(wq_t, TLOC, WN, cq_t, sq_t, "qT")
            wk_t = load_w(wk_d)
            kT = proj_T(wk_t, TEXT, 0, ck_t, sk_t, "kT")

            # ---- v projection (natural orientation, ext rows) ----
            wv_t = load_w(wv_d)
            v_t = sb.tile([P, NKB, D], f32r, tag="v")
            for tc3 in range(NKB):
                rows = P if tc3 < 2 else TEXT - 2 * P
                for half in range(2):
                    pv = pp.tile([P, 512], f32, tag="pp")
                    for k in range(KCH):
                        nc.tensor.matmul(
                            pv[0:rows, :],
                            xT_t[:, k, tc3 * P:tc3 * P + rows],
                            wv_t[:, k, half * 512:(half + 1) * 512],
                            start=(k == 0), stop=(k == KCH - 1),
                        )
                    nc.scalar.activation(
                        v_t[0:rows, tc3, half * 512:(half + 1) * 512],
                        pv[0:rows, :], mybir.ActivationFunctionType.Copy)

            # ---- sliding-window attention ----
            # 3 key blocks (ext rows [128kb, 128kb+128)), all 256 queries;
            # head-outer so psum tiles (mass [1,256]@p0, msg pair@p0/p64)
            # rotate through small pools.
            mass_sb = sb.tile([1, H, TLOC], f32r, tag="mass")
            msg_t = sb.tile([P, MCH, TLOC], f32r, tag="msg")
            for h in range(H):
                po = (h % 2) * HD
                pmass_t = pms.tile([1, TLOC], f32, tag="pmass", name=f"pmass{h}")
                pm_t = pmsg.tile([HD, TLOC], f32, tag="pm", name=f"pm{h}")
                ps_s = psc.tile([P, NKB, TLOC], f32, tag="psc", name=f"ps_{h}")
                for kb in range(NKB):
                    krows = P if kb < 2 else TEXT - 2 * P
                    nc.tensor.matmul(
                        ps_s[0:krows, kb, :],
                        kT[po:po + HD, h // 2, kb * P:kb * P + krows],
                        qT[po:po + HD, h // 2, :],
                        start=True, stop=True,
                    )
                # batched sigmoid + mask (kb0+kb1 full rows; kb2 has 64 rows)
                tau_t = sbt.tile([P, NKB, TLOC], f32r, tag="tau", name=f"tau{h}")
                nc.scalar.activation(tau_t[:, 0:2, :], ps_s[:, 0:2, :],
                                     mybir.ActivationFunctionType.Sigmoid)
                nc.scalar.activation(tau_t[0:HD, 2, :], ps_s[0:HD, 2, :],
                                     mybir.ActivationFunctionType.Sigmoid)
                mask_eng = nc.vector if h % 2 == 0 else nc.gpsimd
                mask_eng.tensor_tensor(tau_t[:, 0:2, :], tau_t[:, 0:2, :],
                                       mask_t[:, 0:2, :],
                                       op=mybir.AluOpType.mult)
                mask_eng.tensor_tensor(tau_t[0:HD, 2, :], tau_t[0:HD, 2, :],
                                       mask_t[0:HD, 2, :],
                                       op=mybir.AluOpType.mult)
                for kb in range(NKB):
                    krows = P if kb < 2 else TEXT - 2 * P
                    nc.tensor.matmul(
                        pmass_t[:], ones_t[0:krows, :], tau_t[0:krows, kb, :],
                        start=(kb == 0), stop=(kb == NKB - 1))
                    nc.tensor.matmul(
                        pm_t[:],
                        v_t[0:krows, kb, h * HD:(h + 1) * HD],
                        tau_t[0:krows, kb, :],
                        start=(kb == 0), stop=(kb == NKB - 1))
                nc.vector.tensor_copy(mass_sb[0:1, h, :], pmass_t[:])
                nc.vector.tensor_copy(msg_t[po:po + HD, h // 2, :], pm_t[:])
                if h % 2 == 1:
                    # divide this head pair's msg chunk immediately so the
                    # wo-projection can be pulled forward by the scheduler
                    c = h // 2
                    pair = mass_sb[0:1, 2 * c:2 * c + 2, :]
                    nc.vector.tensor_scalar_max(pair, pair, 1e-6)
                    with nc.allow_low_precision(
                            reason="f32r rounding of 1/mass is fine"):
                        nc.vector.reciprocal(pair, pair)
                    pb = pp.tile([P, 512], f32, tag="pp", name=f"pb{c}")
                    nc.tensor.matmul(pb[:, 0:TLOC], sel_t[0:1, 0, :],
                                     mass_sb[0:1, 2 * c, :],
                                     start=True, stop=False,
                                     skip_group_check=True)
                    nc.tensor.matmul(pb[:, 0:TLOC], sel_t[0:1, 1, :],
                                     mass_sb[0:1, 2 * c + 1, :],
                                     start=False, stop=True,
                                     skip_group_check=True)
                    nc.vector.tensor_tensor(msg_t[:, c, :], msg_t[:, c, :],
                                            pb[:, 0:TLOC],
                                            op=mybir.AluOpType.mult)

            # ---- output projection + residual + rms norm ----
            wo_t = load_w(wo_d)
            for t2 in range(2):
                z_t = sbr.tile([P, D], f32, tag="z")
                for half in range(2):
                    pz = pp.tile([P, 512], f32, tag="pp")
                    for k in range(KCH):
                        nc.tensor.matmul(
                            pz[:, :],
                            msg_t[:, k, t2 * P:(t2 + 1) * P],
                            wo_t[:, k, half * 512:(half + 1) * 512],
                            start=(k == 0), stop=(k == KCH - 1),
                        )
                    nc.vector.tensor_tensor(
                        z_t[:, half * 512:(half + 1) * 512],
                        pz[:, :], xo_t[:, t2, half * 512:(half + 1) * 512],
                        op=mybir.AluOpType.add)
                z2 = sbr.tile([P, D], f32, tag="zs", name="z2")
                ssq = sbr.tile([P, 1], f32, tag="ssq")
                nc.scalar.activation(z2[:], z_t[:],
                                     mybir.ActivationFunctionType.Square,
                                     accum_out=ssq[:])
                # rms folding: out = z*sqrt(D)/sqrt(ssq+D*eps) * nsc, with
                # sqrt(D) folded into the host-side norm_scale tile.
                nc.vector.tensor_scalar(ssq[:], ssq[:], D * EPS, None,
                                        op0=mybir.AluOpType.add)
                sroot = sbr.tile([P, 1], f32, tag="sroot")
                nc.scalar.activation(sroot[:], ssq[:],
                                     mybir.ActivationFunctionType.Sqrt)
                rinv = sbr.tile([P, 1], f32, tag="rinv")
                nc.vector.reciprocal(rinv[:], sroot[:])
                out_t = sbr.tile([P, D], f32, tag="zs", name="out_t")
                nc.scalar.activation(out_t[:], z_t[:],
                                     mybir.ActivationFunctionType.Copy,
                                     scale=rinv[:])
                nc.gpsimd.tensor_tensor(out_t[:], out_t[:], nsc_t[:],
                                        op=mybir.AluOpType.mult)
                nc.sync.dma_start(y_d[t2 * P:(t2 + 1) * P, :], out_t[:])

    nc.compile()
    return nc


def _host_tables():
    """Core-independent constant inputs."""
    half = HD // 2
    ident = np.eye(P, dtype=np.float32)
    # Rot = blockdiag(J, J) with J = [[0, -I32], [I32, 0]] acting on 64-row groups
    rot = np.zeros((P, P), dtype=np.float32)
    for g in range(2):
        o = g * 64
        for r in range(half):
            rot[o + r, o + half + r] = -1.0
            rot[o + half + r, o + r] = 1.0
    rotT = _round_fp32r(rot.T.copy())
    sel2 = np.zeros((1, 2, P), dtype=np.float32)
    sel2[0, 0, 0:64] = 1.0
    sel2[0, 1, 64:128] = 1.0
    ones_col = np.ones((P, 1), dtype=np.float32)
    return ident, rotT, _round_fp32r(sel2), _round_fp32r(ones_col)


def _trig(positions: np.ndarray, scale: float):
    """cos/sin tables tiled to [128, len(positions)]."""
    half = HD // 2
    theta = 1.0 / (ROPE_BASE ** (np.arange(half, dtype=np.float64) / half))
    freqs = positions[:, None].astype(np.float64) * theta[None, :]  # [n, 32]
    c = (np.cos(freqs).T * scale).astype(np.float32)  # [32, n]
    s = (np.sin(freqs).T * scale).astype(np.float32)
    return np.tile(c, (4, 1)), np.tile(s, (4, 1))


def _masks(core: int) -> np.ndarray:
    """[NKB, 128, TLOC]: mask[kb, j, t] = 1 iff key ext row 128kb+j is in
    query t's window (and causally valid for core 0)."""
    m = np.zeros((NKB, P, TLOC), dtype=np.float32)
    t = np.arange(TLOC)[None, :]
    for kb in range(NKB):
        j = np.arange(P)[:, None]
        w = 128 * kb + j - t
        valid = (w >= 0) & (w < WN)
        if core == 0:
            valid &= (128 * kb + j) >= WN
        m[kb] = valid.astype(np.float32)
    return _round_fp32r(m)


_CACHE = {}


def _make_runner(nc):
    """Persistent sharded-jit executor over the 8 cores (mirrors
    bass2jax.run_bass_via_pjrt's multi-core path, but reusable so repeat
    calls skip retracing/recompilation)."""
    import jax
    from jax.experimental.shard_map import shard_map
    from jax.sharding import Mesh, PartitionSpec
    from concourse import bass2jax

    bass2jax.install_neuronx_cc_hook()
    partition_name = (nc.partition_id_tensor.name
                      if nc.partition_id_tensor else None)
    in_names, out_names, out_avals = [], [], []
    for alloc in nc.m.functions[0].allocations:
        if not isinstance(alloc, mybir.MemoryLocationSet):
            continue
        if alloc.kind not in ("ExternalInput", "ExternalOutput"):
            continue
        name = alloc.memorylocations[0].name
        if alloc.kind == "ExternalInput":
            if name != partition_name:
                in_names.append(name)
        else:
            out_names.append(name)
            out_avals.append(jax.core.ShapedArray(
                tuple(alloc.tensor_shape), mybir.dt.np(alloc.dtype)))
    n_params, n_outs = len(in_names), len(out_names)
    bind_names = in_names + out_names + (
        [partition_name] if partition_name else [])

    def _body(*args):
        operands = list(args)
        if partition_name is not None:
            operands.append(bass2jax.partition_id_tensor())
        outs = bass2jax._bass_exec_p.bind(
            *operands,
            out_avals=tuple(out_avals),
            in_names=tuple(bind_names),
            out_names=tuple(out_names),
            lowering_input_output_aliases=(),
            sim_require_finite=True,
            sim_require_nnan=True,
            nc=nc,
        )
        return tuple(outs)

    devices = jax.devices()[:NCORES]
    mesh = Mesh(np.asarray(devices), ("core",))
    sharded = jax.jit(
        shard_map(_body, mesh=mesh,
                  in_specs=(PartitionSpec("core"),) * (n_params + n_outs),
                  out_specs=(PartitionSpec("core"),) * n_outs,
                  check_rep=False),
        donate_argnums=tuple(range(n_params, n_params + n_outs)),
        keep_unused=True)

    def run(in_maps):
        concat_in = [np.concatenate([m[name] for m in in_maps], axis=0)
                     for name in in_names]
        zeros = [np.zeros((NCORES * a.shape[0], *a.shape[1:]), a.dtype)
                 for a in out_avals]
        out_arrs = sharded(*concat_in, *zeros)
        return [
            {name: np.asarray(out_arrs[i]).reshape(
                NCORES, *out_avals[i].shape)[c]
             for i, name in enumerate(out_names)}
            for c in range(NCORES)
        ]

    run.sharded = sharded
    run.in_names = in_names
    run.out_names = out_names
    run.out_avals = out_avals
    return run


def kernel(x, wq, wk, wv, wo, norm_scale):
    x = np.asarray(x, dtype=np.float32)
    if "nc" not in _CACHE:
        _CACHE["nc"] = _build_program()
        _CACHE["runner"] = _make_runner(_CACHE["nc"])
    nc = _CACHE["nc"]

    ident, rotT, sel2, ones_col = _host_tables()
    wq_r = _round_fp32r(np.asarray(wq))
    wk_r = _round_fp32r(np.asarray(wk))
    wv_r = _round_fp32r(np.asarray(wv))
    wo_r = _round_fp32r(np.asarray(wo))
    nsc = np.ascontiguousarray(
        np.broadcast_to(np.asarray(norm_scale, dtype=np.float32)
                        * np.float32(np.sqrt(D)), (P, D)))

    xf = x.reshape(T, D)
    in_maps = []
    for c in range(NCORES):
        t0 = c * TLOC
        x_halo = np.zeros((TEXT, D), dtype=np.float32)
        lo = max(0, t0 - WN)
        x_halo[WN - (t0 - lo):] = xf[lo:t0 + TLOC]
        cosq, sinq = _trig(np.arange(t0, t0 + TLOC), 1.0 / 8.0)
        cosk, sink = _trig(np.arange(t0 - WN, t0 + TLOC), 1.0)
        in_maps.append({
            "x_halo": x_halo,
            "x_own": np.ascontiguousarray(xf[t0:t0 + TLOC]),
            "wq_r": wq_r, "wk_r": wk_r, "wv_r": wv_r, "wo_r": wo_r,
            "ident": ident, "rotT": rotT,
            "cosq": cosq, "sinq": sinq, "cosk": cosk, "sink": sink,
            "masks": _masks(c), "sel2": sel2, "ones_col": ones_col,
            "norm_scale": nsc,
        })

    _CACHE["last_in_maps"] = in_maps
    if "first_done" not in _CACHE:
        res = bass_utils.run_bass_kernel_spmd(
            nc, in_maps, core_ids=list(range(NCORES)))
        results = res.results
        _CACHE["first_done"] = True
    else:
        results = _CACHE["runner"](in_maps)
    out = np.empty((1, T, D), dtype=np.float32)
    for c in range(NCORES):
        out[0, c * TLOC:(c + 1) * TLOC] = results[c]["y"]
    return out

